# revision 1
# baseline (speedup 1.0000x reference)
"""Trainium2 Bass kernel for the 2D-attention module (nn_Attention2D).

Reference computation (per batch element b):
    g_em   = img_fvec @ W1.T + b1                       # [HID]
    x_em   = conv3x3_same(patch_fmap, conv_w) + conv_b  # [HID, H, W]
    actv   = tanh(x_em + g_em[:, None, None])           # [HID, H, W]
    logits = W2 @ actv.reshape(HID, HW)                 # [1, HW]  (+b2, softmax-invariant)
    wts    = softmax(logits)                            # [1, HW]
    attn   = patch_fmap.reshape(C, HW) @ wts.T          # [C]

Sharding: pure data parallel, 8 images per core on 8 cores; weights replicated.

Per-core device program (channel-on-partition layout):
  - conv as 9 shifted matmuls over a zero-padded [128, 30, 30] bf16 buffer per
    C_in chunk; 36 accumulating matmuls per PSUM group [128 c_out, 392 pos].
  - tanh on ScalarE with per-partition bias = g_em[b] + b1 + conv_b.
  - logits via M=1 matmuls contracted over c_out chunks.
  - softmax on a single partition (DVE+ACT), normalized in SBUF.
  - partition-broadcast of normalized softmax weights via a DRAM bounce and a
    0-stride-partition SWDGE (gpsimd) DMA read-back.
  - weighted feature sum: DVE tensor_mul (patch_f32 * e) then ScalarE
    Identity-activation with accum_out -> attn[128, 1] per C_in chunk.
"""

import numpy as np
import ml_dtypes

import concourse.bass as bass
import concourse.bacc as bacc
import concourse.tile as tile
from concourse import mybir
from concourse.bass_utils import run_bass_kernel_spmd

# Problem shapes (hardcoded; kernel.py must be self-contained).
B = 64
C_IN = 512
HID = 512
H = W = 28
HW = H * W            # 784
N_CORES = 8
B_PER_CORE = B // N_CORES  # 8
KC = C_IN // 128      # 4 k chunks (contraction over c_in)
MC = HID // 128       # 4 m chunks (c_out partitions)
NH = 2                # spatial halves (14 rows x 28 cols = 392 <= 512 PSUM bank)
NHALF = HW // NH      # 392
ROWS_PER_HALF = H // NH  # 14

FP32 = mybir.dt.float32
BF16 = mybir.dt.bfloat16


def build_bass():
    nc = bacc.Bacc(None)

    # Per-core inputs.
    patch_d = nc.dram_tensor("patch", [B_PER_CORE, C_IN, H, W], FP32,
                             kind="ExternalInput")
    imgT_d = nc.dram_tensor("imgT", [C_IN, B_PER_CORE], BF16,
                            kind="ExternalInput")
    w1t_d = nc.dram_tensor("w1t", [C_IN, HID], BF16, kind="ExternalInput")
    convwt_d = nc.dram_tensor("convwt", [9, C_IN, HID], BF16,
                              kind="ExternalInput")
    w2_d = nc.dram_tensor("w2", [HID], BF16, kind="ExternalInput")
    bsum_d = nc.dram_tensor("bsum", [HID], FP32, kind="ExternalInput")
    # Output laid out to match attn_sb exactly ([partition, k, b]) so the
    # final DMA is a single contiguous copy; the host transposes.
    out_d = nc.dram_tensor("out", [128, KC, B_PER_CORE], FP32,
                           kind="ExternalOutput")

    with tile.TileContext(nc) as tc:
        with (
            tc.tile_pool(name="wpool", bufs=1) as wpool,
            tc.tile_pool(name="pfpool", bufs=16) as pfpool,
            tc.tile_pool(name="padpool", bufs=10) as padpool,
            tc.tile_pool(name="actvpool", bufs=3) as actvpool,
            tc.tile_pool(name="spool", bufs=4) as spool,
            tc.tile_pool(name="scrpool", bufs=3) as scrpool,
            tc.tile_pool(name="ebspool", bufs=3) as ebspool,
            tc.tile_pool(name="dpool", bufs=4, space="DRAM") as dpool,
            tc.tile_pool(name="cpool", bufs=6, space="PSUM") as cpool,
            tc.tile_pool(name="lpool", bufs=1, space="PSUM") as lpool,
        ):
            # ---- Preload weights/constants ----
            # Small tensors first, then image-0 patches, then conv weights
            # per tap so the first conv matmuls can start early.
            w1t_sb = wpool.tile([128, KC, HID], BF16)
            nc.sync.dma_start(
                out=w1t_sb, in_=w1t_d[:].rearrange("(k p) c -> p k c", p=128)
            )
            imgT_sb = wpool.tile([128, KC, B_PER_CORE], BF16)
            nc.sync.dma_start(
                out=imgT_sb, in_=imgT_d[:].rearrange("(k p) b -> p k b", p=128)
            )
            w2_sb = wpool.tile([128, MC], BF16)
            nc.sync.dma_start(
                out=w2_sb, in_=w2_d[:].rearrange("(k p) -> p k", p=128)
            )
            bsum_sb = wpool.tile([128, MC], FP32)
            nc.sync.dma_start(
                out=bsum_sb, in_=bsum_d[:].rearrange("(k p) -> p k", p=128)
            )
            convwt_sb = wpool.tile([128, 9 * KC, HID], BF16)
            # ---- g_em for all images: gbias[c_out, m, b] = W1@img + b1 + conv_b
            gbias_sb = wpool.tile([128, MC, B_PER_CORE], FP32)
            for m in range(MC):
                gps = cpool.tile([128, B_PER_CORE], FP32, tag="cps")
                for k in range(KC):
                    nc.tensor.matmul(
                        gps,
                        w1t_sb[:, k, m * 128:(m + 1) * 128],
                        imgT_sb[:, k, :],
                        start=(k == 0),
                        stop=(k == KC - 1),
                    )
                nc.scalar.activation(
                    out=gbias_sb[:, m, :],
                    in_=gps,
                    func=mybir.ActivationFunctionType.Identity,
                    bias=bsum_sb[:, m:m + 1],
                    scale=1.0,
                )

            # ---- Per-image pipeline ----
            state = {}  # image index -> tiles produced/needed per stage

            def emit_loads(b):
                padded = []
                pfs = []
                for k in range(KC):
                    pf = pfpool.tile([128, H, W], FP32, tag="pf")
                    nc.sync.dma_start(
                        out=pf, in_=patch_d[b, k * 128:(k + 1) * 128, :, :]
                    )
                    pfs.append(pf)
                    pad = padpool.tile([128, H + 2, W + 2], BF16, tag="pad")
                    nc.gpsimd.memset(pad, 0.0)
                    nc.scalar.copy(out=pad[:, 1:H + 1, 1:W + 1], in_=pf)
                    padded.append(pad)
                state[b] = {"pfs": pfs, "padded": padded}

            def emit_conv(b, tap_streaming=False):
                padded = state[b]["padded"]
                actv = actvpool.tile([128, MC, HW], BF16, tag="actv")
                state[b]["actv"] = actv

                def tanh_out(cps, m, h):
                    nc.scalar.activation(
                        out=actv[:, m, h * NHALF:(h + 1) * NHALF],
                        in_=cps,
                        func=mybir.ActivationFunctionType.Tanh,
                        bias=gbias_sb[:, m, b:b + 1],
                        scale=1.0,
                    )

                if tap_streaming:
                    # Image 0: taps are still streaming from HBM. Consume
                    # them tap-outer across 4 concurrent PSUM groups so the
                    # PE never waits for a late tap.
                    for h in range(NH):
                        cps_l = [cpool.tile([128, NHALF], FP32, tag="cps",
                                            name=f"cps0_{h}_{m}")
                                 for m in range(MC)]
                        for t in range(9):
                            dy, dx = t // 3, t % 3
                            r0 = h * ROWS_PER_HALF + dy
                            for k in range(KC):
                                for m in range(MC):
                                    nc.tensor.matmul(
                                        cps_l[m],
                                        convwt_sb[:, t * KC + k,
                                                  m * 128:(m + 1) * 128],
                                        padded[k][:, r0:r0 + ROWS_PER_HALF,
                                                  dx:dx + W],
                                        start=(t == 0 and k == 0),
                                        stop=(t == 8 and k == KC - 1),
                                    )
                        for m in range(MC):
                            tanh_out(cps_l[m], m, h)
                    return

                for m in range(MC):
                    for h in range(NH):
                        cps = cpool.tile([128, NHALF], FP32, tag="cps")
                        first = True
                        for t in range(9):
                            dy, dx = t // 3, t % 3
                            r0 = h * ROWS_PER_HALF + dy
                            for k in range(KC):
                                nc.tensor.matmul(
                                    cps,
                                    convwt_sb[:, t * KC + k,
                                              m * 128:(m + 1) * 128],
                                    padded[k][:, r0:r0 + ROWS_PER_HALF,
                                              dx:dx + W],
                                    start=first,
                                    stop=(t == 8 and k == KC - 1),
                                )
                                first = False
                        tanh_out(cps, m, h)

            def emit_finale1(b):
                """logits -> softmax -> normalized weights -> DRAM bounce."""
                actv = state[b]["actv"]
                lps = lpool.tile([1, NH, 512], FP32, tag="lps")
                for h in range(NH):
                    for m in range(MC):
                        nc.tensor.matmul(
                            lps[:, h, 0:NHALF],
                            w2_sb[:, m:m + 1],
                            actv[:, m, h * NHALF:(h + 1) * NHALF],
                            start=(m == 0),
                            stop=(m == MC - 1),
                        )
                # softmax on partition 0
                negmax = spool.tile([1, 1], FP32, tag="negmax")
                nc.vector.reduce_max(
                    out=negmax, in_=lps[:, :, 0:NHALF],
                    axis=mybir.AxisListType.XY, negate=True,
                )
                e_sb = spool.tile([1, HW], FP32, tag="e_sb")
                nc.scalar.activation(
                    out=e_sb.rearrange("p (h n) -> p h n", h=NH),
                    in_=lps[:, :, 0:NHALF],
                    func=mybir.ActivationFunctionType.Exp,
                    bias=negmax,
                    scale=1.0,
                )
                ssum = spool.tile([1, 1], FP32, tag="ssum")
                nc.vector.reduce_sum(out=ssum, in_=e_sb,
                                     axis=mybir.AxisListType.X)
                rsum = spool.tile([1, 1], FP32, tag="rsum")
                nc.vector.reciprocal(out=rsum, in_=ssum)
                en_sb = spool.tile([1, HW], FP32, tag="en_sb")
                nc.vector.tensor_scalar_mul(en_sb, e_sb, rsum)

                # stage the normalized weights in DRAM for partition-broadcast
                escr = dpool.tile([1, HW], FP32, tag="escr")
                nc.sync.dma_start(out=escr, in_=en_sb)
                state[b]["escr"] = escr

            def emit_finale2(b):
                """0-stride-partition SWDGE broadcast + weighted feature sum."""
                st = state.pop(b)
                escr = st["escr"]
                ebs = ebspool.tile([128, HW], FP32, tag="ebs")
                nc.gpsimd.dma_start(
                    out=ebs,
                    in_=bass.AP(tensor=escr.tensor, offset=escr.offset,
                                ap=[[0, 128], [1, HW]]),
                )
                for k in range(KC):
                    scr = scrpool.tile([128, HW], FP32, tag="scr")
                    nc.vector.tensor_tensor(
                        out=scr,
                        in0=st["pfs"][k].rearrange("p a b -> p (a b)"),
                        in1=ebs,
                        op=mybir.AluOpType.mult,
                    )
                    nc.scalar.activation(
                        out=scr,
                        in_=scr,
                        func=mybir.ActivationFunctionType.Identity,
                        accum_out=attn_sb[:, k, b:b + 1],
                    )

            attn_sb = wpool.tile([128, KC, B_PER_CORE], FP32)
            emit_loads(0)
            for t in range(9):
                nc.sync.dma_start(
                    out=convwt_sb[:, t * KC:(t + 1) * KC, :],
                    in_=convwt_d[t].rearrange("(k p) c -> p k c", p=128),
                )
            for b in range(B_PER_CORE):
                if b + 1 < B_PER_CORE:
                    emit_loads(b + 1)
                emit_conv(b, tap_streaming=(b == 0))
                if b >= 1:
                    emit_finale1(b - 1)
                if b >= 2:
                    emit_finale2(b - 2)
            emit_finale2(B_PER_CORE - 2)
            emit_finale1(B_PER_CORE - 1)
            emit_finale2(B_PER_CORE - 1)

            nc.sync.dma_start(out=out_d[:], in_=attn_sb)

    nc.compile()
    return nc


_CACHED = {}


def get_bass():
    if "nc" not in _CACHED:
        _CACHED["nc"] = build_bass()
    return _CACHED["nc"]


def make_in_maps(img_fvec, patch_fmap, W1, b1, conv_w, conv_b, W2, b2):
    img_fvec = np.asarray(img_fvec, dtype=np.float32)
    patch_fmap = np.ascontiguousarray(np.asarray(patch_fmap, dtype=np.float32))
    W1 = np.asarray(W1, dtype=np.float32)
    b1 = np.asarray(b1, dtype=np.float32)
    conv_w = np.asarray(conv_w, dtype=np.float32)
    conv_b = np.asarray(conv_b, dtype=np.float32)
    W2 = np.asarray(W2, dtype=np.float32)
    # b2 shifts every logit equally; softmax is shift-invariant, so it drops out.

    w1t = np.ascontiguousarray(W1.T).astype(ml_dtypes.bfloat16)
    convwt = np.ascontiguousarray(
        conv_w.transpose(2, 3, 1, 0).reshape(9, C_IN, HID)
    ).astype(ml_dtypes.bfloat16)
    w2 = np.ascontiguousarray(W2[0]).astype(ml_dtypes.bfloat16)
    bsum = np.ascontiguousarray(b1 + conv_b).astype(np.float32)

    in_maps = []
    for c in range(N_CORES):
        sl = slice(c * B_PER_CORE, (c + 1) * B_PER_CORE)
        imgT = np.ascontiguousarray(img_fvec[sl].T).astype(ml_dtypes.bfloat16)
        in_maps.append({
            "patch": np.ascontiguousarray(patch_fmap[sl]),
            "imgT": imgT,
            "w1t": w1t,
            "convwt": convwt,
            "w2": w2,
            "bsum": bsum,
        })
    return in_maps


def kernel(img_fvec, patch_fmap, W1, b1, conv_w, conv_b, W2, b2,
           trace=False, **run_kwargs):
    nc = get_bass()
    in_maps = make_in_maps(img_fvec, patch_fmap, W1, b1, conv_w, conv_b,
                           W2, b2)
    res = run_bass_kernel_spmd(nc, in_maps, core_ids=list(range(N_CORES)),
                               trace=trace, **run_kwargs)
    # per-core result is [p, k, b] -> [b, k*128+p]
    out = np.concatenate(
        [r["out"].transpose(2, 1, 0).reshape(B_PER_CORE, C_IN)
         for r in res.results], axis=0)
    if trace:
        kernel.last_results = res
    return out

